# revision 14
# baseline (speedup 1.0000x reference)
"""Trainium2 Bass kernel for the prototypical-network resampling loss.

Distribution: data-parallel over the 40000 queries across 8 NeuronCores
(5000 queries/core).  Per-class pseudo-label statistics are combined with
an in-kernel AllReduce; the tiny [5,640] protos are replicated.  The final
loss mean is finished on the host from 8 per-core partial sums.
"""

import sys

import numpy as np

for _p in ("/opt/trn_rl_repo", "/root/.axon_site/_ro/trn_rl_repo"):
    if _p not in sys.path:
        sys.path.insert(0, _p)

import concourse.bass as bass
import concourse.masks as masks
import concourse.mybir as mybir
import concourse.tile as tile
from bass_rust import ScopedClock
from concourse.bass_utils import run_bass_kernel_spmd

F32 = mybir.dt.float32
F32R = mybir.dt.float32r

# Problem constants (hardcoded per harness contract).
Q, D, K, NSHOT, NSAMP = 40000, 640, 5, 5, 50
S = K * NSHOT  # 25 support rows
NCORES = 8
P = 128
DCH = D // P  # 5 partition-chunks of the feature dim
GQ = 512      # queries per matmul group (fp32r needs N>=256)

# Use fp32r (full-rate PE) for the distance & stats matmuls.
DIST_F32R = True
STATS_F32R = True


def _ceil_div(a, b):
    return (a + b - 1) // b


class _TileContextSplitWaits(tile.TileContext):
    """TileContext whose lowering splits >1 sem-waits per instruction.

    The walrus build in this container rejects instructions carrying more
    than one sync-wait command, while Tile freely attaches several.  After
    normal lowering we walk every basic block and move extra waits onto
    freshly created same-engine NoOps placed immediately before the
    over-subscribed instruction (in-order engines make this equivalent).
    """

    def _drain_and_barrier(self, tick_clock, wait_clock):
        nc = self.nc
        drain_inst = nc.sync.drain()
        wait_clock.add_sem_waits(
            drain_inst.ins, ScopedClock({None: tick_clock.global_clock})
        )
        nc.all_engine_barrier()
        assert self.sems is not None
        popped = nc._tile_sem_poison_stack.pop()
        assert popped is self._sem_poison
        nc.clear_and_free_semaphores(list(self.sems.allocated().values()))
        nc.all_engine_barrier()
        self._split_multi_waits()

    def _split_multi_waits(self):
        nc = self.nc
        snapshots = [(bb, list(bb.instructions)) for bb in nc.main_func.blocks]
        for bb, insts in snapshots:
            new_list = []
            changed = False
            for inst in insts:
                si = inst.sync_info
                if si is not None and si.on_wait and len(si.on_wait) > 1:
                    changed = True
                    waits = list(si.on_wait)
                    for w in waits[:-1]:
                        nop = nc.engines[inst.engine].nop()
                        nop.ins.sync_info = mybir.SyncInfo(on_wait=[w], on_update=[])
                        new_list.append(nop.ins)
                    inst.sync_info = mybir.SyncInfo(
                        on_wait=[waits[-1]], on_update=list(si.on_update or [])
                    )
                new_list.append(inst)
            if changed:
                bb.instructions = new_list


def _build_program(qc):
    return _build_program_reps(qc, 1)


def _build_program_reps(qc, reps):
    """Build the SPMD Bass program; body emitted `reps` times (for timing)."""
    nt = _ceil_div(qc, P)
    nc = bass.Bass()
    params = dict(
        qin=nc.declare_dram_parameter("q", [qc, D], F32R, isOutput=False),
        lbl_in=nc.declare_dram_parameter("lblhot", [P, nt * K], F32, isOutput=False),
        we_in=nc.declare_dram_parameter("wE", [P, DCH * K], F32R, isOutput=False),
        ppe_in=nc.declare_dram_parameter("ppE", [1, K], F32R, isOutput=False),
        ssum_in=nc.declare_dram_parameter("ssumT", [P, DCH * K], F32, isOutput=False),
        ssq_in=nc.declare_dram_parameter("ssqT", [P, DCH * K], F32, isOutput=False),
        ss55_in=nc.declare_dram_parameter("ss55T", [P, DCH * K], F32, isOutput=False),
        ns55_in=nc.declare_dram_parameter("nsum55T", [P, DCH * K], F32, isOutput=False),
        y_out=nc.declare_dram_parameter("y", [P, nt * K], F32, isOutput=True),
        loss_out=nc.declare_dram_parameter("losspart", [1, 1], F32, isOutput=True),
    )
    for _ in range(reps):
        _emit_body(nc, qc, params)
    return nc


def _emit_body(nc, qc, params):
    nt = _ceil_div(qc, P)                      # number of 128-query tiles
    ngroups = _ceil_div(qc, GQ)
    qin = params["qin"]
    lbl_in = params["lbl_in"]
    we_in = params["we_in"]
    ppe_in = params["ppe_in"]
    ssum_in = params["ssum_in"]
    ssq_in = params["ssq_in"]
    ss55_in = params["ss55_in"]
    ns55_in = params["ns55_in"]
    y_out = params["y_out"]
    loss_out = params["loss_out"]

    def r(ap):
        return ap.bitcast(F32R)

    def groups():
        out = []
        for g in range(ngroups):
            q0 = g * GQ
            glen = min(GQ, qc - q0)
            subs = []
            off = 0
            while off < glen:
                tp = min(P, glen - off)
                subs.append((q0 + off, off // P, tp))
                off += tp
            out.append((q0, glen, subs))
        return out

    with _TileContextSplitWaits(nc) as tc:
        with (
            tc.tile_pool(name="per", bufs=1) as per,       # persistent sbuf
            tc.tile_pool(name="rot", bufs=3) as rot,       # rotating loads
            tc.tile_pool(name="small", bufs=2) as small,   # rotating small sbuf
            tc.tile_pool(name="dram", bufs=1, space="DRAM") as dram,
        ):
            # ---------------- persistent SBUF ----------------
            ident = per.tile([P, P], F32, tag="ident")
            masks.make_identity(nc, ident[:])
            ones_f = per.tile([1, GQ], F32, tag="ones_f")
            nc.vector.memset(ones_f[:], 1.0)
            ones_row = per.tile([1, GQ], F32R, tag="ones_row")
            nc.vector.tensor_copy(ones_row[:], ones_f[:])
            ones_col = per.tile([P, 1], F32, tag="ones_col")
            nc.vector.memset(ones_col[:], 1.0)
            ones4 = per.tile([P, 8], F32, tag="ones4")
            nc.vector.memset(ones4[:], 1.0)

            qT = per.tile([P, DCH, qc], F32R, tag="qT")
            qq_all = per.tile([P, nt], F32, tag="qq_all")
            mn_all = per.tile([P, nt], F32, tag="mn_all")
            s_all = per.tile([P, nt], F32, tag="s_all")
            dl_all = per.tile([P, nt], F32, tag="dl_all")
            y_all = per.tile([P, nt, K], F32, tag="y_all")
            lbl_sb = per.tile([P, nt, K], F32, tag="lbl_sb")
            we_sb = per.tile([P, DCH, K], F32R, tag="we_sb")
            ppe_sb = per.tile([1, K], F32R, tag="ppe_sb")
            ssum_sb = per.tile([P, DCH, K], F32, tag="ssum_sb")
            ssq_sb = per.tile([P, DCH, K], F32, tag="ssq_sb")
            ss55_sb = per.tile([P, DCH, K], F32, tag="ss55_sb")
            ns55_sb = per.tile([P, DCH, K], F32, tag="ns55_sb")

            # runt-tile columns must read as neutral values in the loss
            nc.vector.memset(mn_all[:], 0.0)
            nc.vector.memset(s_all[:], 1.0)
            nc.vector.memset(dl_all[:], 0.0)
            nc.vector.memset(y_all[:], 0.0)

            nc.sync.dma_start(lbl_sb[:].rearrange("p t k -> p (t k)"), lbl_in[:])
            nc.sync.dma_start(we_sb[:].rearrange("p c k -> p (c k)"), we_in[:])
            nc.sync.dma_start(ppe_sb[:], ppe_in[:])
            nc.sync.dma_start(ssum_sb[:].rearrange("p c k -> p (c k)"), ssum_in[:])
            nc.sync.dma_start(ssq_sb[:].rearrange("p c k -> p (c k)"), ssq_in[:])
            nc.sync.dma_start(ss55_sb[:].rearrange("p c k -> p (c k)"), ss55_in[:])
            nc.sync.dma_start(ns55_sb[:].rearrange("p c k -> p (c k)"), ns55_in[:])

            ar_in = per.tile([K, 2 * D + 2], F32, tag="ar_in")
            st_sb = per.tile([K, 2 * D + 2], F32, tag="st_sb")

            ib_stats = dram.tile([K, 2 * D + 2], F32)
            ob_stats = dram.tile([K, 2 * D + 2], F32)

            # ---------------- phase 1: stream queries ----------------
            with (
                tc.tile_pool(name="pstat", bufs=1, space="PSUM") as pstat,
                tc.tile_pool(name="pstage", bufs=1, space="PSUM") as pstage,
                tc.tile_pool(name="pdist", bufs=1, space="PSUM") as pdist,
            ):
                qsum_a = pstat.tile([K, 320], F32, tag="qsum_a")
                qsum_b = pstat.tile([K, 322], F32, tag="qsum_b")
                qsq_a = pstat.tile([K, 320], F32, tag="qsq_a")
                qsq_b = pstat.tile([K, 320], F32, tag="qsq_b")

                first_tile = True
                last_q0 = groups()[-1][2][-1][0]
                for q0, glen, subs in groups():
                    nsub = len(subs)
                    qg = rot.tile([P, nsub, D + 2], F32R, tag="qg")
                    qsqg = rot.tile([P, nsub, D], F32R, tag="qsqg")
                    # load queries [q0 : q0+glen] and a ones column
                    nfull = sum(1 for (_, _, tp) in subs if tp == P)
                    if nfull:
                        nc.sync.dma_start(
                            qg[:, :nfull, :D],
                            qin[q0 : q0 + nfull * P, :].rearrange(
                                "(j p) d -> p j d", p=P
                            ),
                        )
                    for qs, j, tp in subs:
                        if tp != P:
                            nc.sync.dma_start(
                                qg[:tp, j, :D], qin[qs : qs + tp, :]
                            )
                    nc.vector.tensor_copy(
                        qg[:, :, D : D + 2],
                        ones4[:, : 2 * nsub].rearrange("p (t k) -> p t k", k=2),
                    )

                    psD = pdist.tile([K, GQ], F32, tag="psD")
                    psT = pstage.tile([P, D], F32, tag="psT")
                    d2 = pdist.tile([P, nsub, K], F32, tag="d2")

                    for qs, j, tp in subs:
                        t = qs // P
                        # qT chunks via PE transpose, staged in PSUM
                        qgf = qg[:].bitcast(F32)
                        for c in range(DCH):
                            nc.tensor.transpose(
                                psT[:, c * P : c * P + tp],
                                qgf[:tp, j, c * P : (c + 1) * P],
                                ident[:tp, :tp],
                            )
                        nc.vector.tensor_copy(
                            qT[:, :, qs : qs + tp],
                            psT[:].rearrange("p (c u) -> p c u", c=DCH)[:, :, :tp],
                        )
                        # squares + per-query norms on ACT
                        nc.scalar.activation(
                            qsqg[:tp, j, :],
                            qg[:].bitcast(F32)[:tp, j, :D],
                            mybir.ActivationFunctionType.Square,
                            accum_out=qq_all[:tp, t : t + 1],
                        )

                    # dist_e^lin = -2 q.p + ||p||^2, transposed [K, glen]
                    wer, qTr = we_sb[:], qT[:]
                    oner, pper = ones_row[:], ppe_sb[:]
                    for c in range(DCH):
                        nc.tensor.matmul(
                            psD[:, :glen],
                            wer[:, c, :],
                            qTr[:, c, q0 : q0 + glen],
                            start=(c == 0),
                            stop=False,
                        )
                    nc.tensor.matmul(
                        psD[:, :glen],
                        pper,
                        oner[:, :glen],
                        start=False,
                        stop=True,
                    )
                    # transpose back to [tp, K] per subtile
                    dT_sb = small.tile([K, GQ], F32, tag="dT")
                    nc.scalar.copy(dT_sb[:, :glen], psD[:, :glen])
                    for qs, j, tp in subs:
                        nc.tensor.transpose(
                            d2[:tp, j, :],
                            dT_sb[:, j * P : j * P + tp],
                            ident[:K, :K],
                        )

                    # ---- per-query math over the group ----
                    dist = small.tile([P, nsub, K], F32, tag="dist")
                    oh = small.tile([P, nsub, K], F32R, tag="oh")
                    e_sb = small.tile([P, nsub, K], F32, tag="e_sb")
                    scr = small.tile([P, nsub, K], F32, tag="scr")

                    def batch(sl_p, sl_j, tspan):
                        """per-query ops on partitions sl_p, subtiles sl_j"""
                        pz, jz = sl_p, sl_j
                        nj = jz.stop - jz.start
                        qqb = (
                            qq_all[pz, tspan]
                            .unsqueeze(2)
                            .broadcast_to([pz.stop - pz.start, nj, K])
                        )
                        nc.vector.scalar_tensor_tensor(
                            scr[pz, jz, :],
                            d2[pz, jz, :],
                            1.0,
                            qqb,
                            mybir.AluOpType.mult,
                            mybir.AluOpType.add,
                        )
                        nc.scalar.activation(
                            dist[pz, jz, :],
                            scr[pz, jz, :],
                            mybir.ActivationFunctionType.Sqrt,
                        )
                        nc.vector.tensor_reduce(
                            mn_all[pz, tspan],
                            dist[pz, jz, :],
                            mybir.AxisListType.X,
                            mybir.AluOpType.min,
                        )
                        mnb = (
                            mn_all[pz, tspan]
                            .unsqueeze(2)
                            .broadcast_to([pz.stop - pz.start, nj, K])
                        )
                        nc.vector.tensor_tensor(
                            oh[pz, jz, :], dist[pz, jz, :], mnb,
                            mybir.AluOpType.is_equal,
                        )
                        nc.scalar.activation(
                            e_sb[pz, jz, :],
                            dist[pz, jz, :],
                            mybir.ActivationFunctionType.Exp,
                            scale=-1.0,
                        )
                        nc.vector.tensor_reduce(
                            s_all[pz, tspan],
                            e_sb[pz, jz, :],
                            mybir.AxisListType.X,
                            mybir.AluOpType.add,
                        )
                        nc.vector.tensor_tensor(
                            scr[pz, jz, :], dist[pz, jz, :], lbl_sb[pz, tspan, :],
                            mybir.AluOpType.mult,
                        )
                        nc.vector.tensor_reduce(
                            dl_all[pz, tspan],
                            scr[pz, jz, :],
                            mybir.AxisListType.X,
                            mybir.AluOpType.add,
                        )

                    t0 = q0 // P
                    nfull = sum(1 for (_, _, tp) in subs if tp == P)
                    if nfull:
                        batch(slice(0, P), slice(0, nfull), slice(t0, t0 + nfull))
                    for qs, j, tp in subs:
                        if tp != P:
                            batch(slice(0, tp), slice(j, j + 1),
                                  slice(qs // P, qs // P + 1))

                    # ---- pseudo-label statistics matmuls ----
                    for qs, j, tp in subs:
                        ohap = oh[:tp, j, :]
                        qgap = qg[:tp, j, :]
                        qsqap = qsqg[:tp, j, :]
                        last = qs == last_q0
                        nc.tensor.matmul(qsum_a[:], ohap, qgap[:, :320],
                                         start=first_tile, stop=last)
                        nc.tensor.matmul(qsum_b[:], ohap, qgap[:, 320:642],
                                         start=first_tile, stop=last)
                        nc.tensor.matmul(qsq_a[:], ohap, qsqap[:, :320],
                                         start=first_tile, stop=last)
                        nc.tensor.matmul(qsq_b[:], ohap, qsqap[:, 320:640],
                                         start=first_tile, stop=last)
                        first_tile = False

                # stats -> SBUF -> DRAM -> AllReduce
                nc.scalar.copy(ar_in[:, 0:320], qsum_a[:])
                nc.scalar.copy(ar_in[:, 320:642], qsum_b[:])
                nc.scalar.copy(ar_in[:, 642:962], qsq_a[:])
                nc.scalar.copy(ar_in[:, 962:1282], qsq_b[:])

            nc.sync.dma_start(ib_stats[:], ar_in[:])
            nc.gpsimd.collective_compute(
                "AllReduce",
                mybir.AluOpType.add,
                replica_groups=[list(range(NCORES))],
                ins=[ib_stats.opt()],
                outs=[ob_stats.opt()],
            )
            nc.sync.dma_start(st_sb[:], ob_stats[:])

            # ---------------- phase 2: new protos ----------------
            with (
                tc.tile_pool(name="p2a", bufs=1, space="PSUM") as p2a,
                tc.tile_pool(name="p2d", bufs=2, space="PSUM") as p2d,
            ):
                # q_sum/q_sqsum transposed into [p, c, k] layout; cnt row
                pstatT = p2a.tile([P, 2 * DCH * K + K], F32, tag="pstatT")
                for c in range(DCH):
                    nc.tensor.transpose(
                        pstatT[:, c * K : (c + 1) * K],
                        st_sb[:, c * P : (c + 1) * P],
                        ident[:K, :K],
                    )
                    nc.tensor.transpose(
                        pstatT[:, (DCH + c) * K : (DCH + c + 1) * K],
                        st_sb[:, 642 + c * P : 642 + (c + 1) * P],
                        ident[:K, :K],
                    )
                nc.tensor.transpose(
                    pstatT[:1, 2 * DCH * K : 2 * DCH * K + K],
                    st_sb[:, 640:641],
                    ident[:K, :K],
                )
                statT = per.tile([P, 2 * DCH * K + K], F32, tag="statT")
                nc.scalar.copy(statT[:, : 2 * DCH * K], pstatT[:, : 2 * DCH * K])
                nc.scalar.copy(
                    statT[0:1, 2 * DCH * K :], pstatT[0:1, 2 * DCH * K :]
                )

                qsumT = statT[:, : DCH * K].rearrange("p (c k) -> p c k", c=DCH)
                qsqT = statT[:, DCH * K : 2 * DCH * K].rearrange(
                    "p (c k) -> p c k", c=DCH
                )
                cnt_row = statT[0:1, 2 * DCH * K : 2 * DCH * K + K]

                # per-class scalars as [1, K] rows, then broadcast via PE
                # reference counts include the NSHOT support points per class
                cntN = per.tile([1, K], F32, tag="cntN")
                nc.vector.tensor_scalar_add(cntN[:], cnt_row, float(NSHOT))
                rowbuf = per.tile([1, 2 * K], F32, tag="rowbuf")
                nc.vector.reciprocal(rowbuf[:, 0:K], cntN[:])
                cm1 = per.tile([1, K], F32, tag="cm1")
                nc.vector.tensor_scalar_add(cm1[:], cnt_row, float(NSHOT) - 1.0)
                rc1 = per.tile([1, K], F32, tag="rc1")
                nc.vector.reciprocal(rc1[:], cm1[:])
                nc.vector.tensor_tensor(
                    rowbuf[:, K : 2 * K], cntN[:], rc1[:], mybir.AluOpType.mult
                )
                pbc = p2a.tile([P, 2 * K], F32, tag="pbc")
                nc.tensor.matmul(pbc[:], ones_row[:, :P].bitcast(F32), rowbuf[:],
                                 start=True, stop=True)
                bc_sb = per.tile([P, 2 * K], F32, tag="bc_sb")
                nc.scalar.copy(bc_sb[:], pbc[:])
                rcB = bc_sb[:, 0:K].unsqueeze(1).broadcast_to([P, DCH, K])
                fB = bc_sb[:, K : 2 * K].unsqueeze(1).broadcast_to([P, DCH, K])

                meanT = per.tile([P, DCH, K], F32, tag="meanT")
                nc.vector.tensor_tensor(
                    meanT[:], qsumT, ssum_sb[:], mybir.AluOpType.add
                )
                nc.vector.tensor_tensor(meanT[:], meanT[:], rcB,
                                        mybir.AluOpType.mult)
                enT = per.tile([P, DCH, K], F32, tag="enT")
                nc.vector.tensor_tensor(enT[:], qsqT, ssq_sb[:],
                                        mybir.AluOpType.add)
                nc.vector.tensor_tensor(enT[:], enT[:], rcB, mybir.AluOpType.mult)
                m2 = per.tile([P, DCH, K], F32, tag="m2")
                nc.vector.tensor_tensor(m2[:], meanT[:], meanT[:],
                                        mybir.AluOpType.mult)
                varT = per.tile([P, DCH, K], F32, tag="varT")
                nc.vector.tensor_tensor(varT[:], enT[:], m2[:],
                                        mybir.AluOpType.subtract)
                nc.vector.tensor_tensor(varT[:], varT[:], fB, mybir.AluOpType.mult)
                nc.vector.tensor_scalar_max(varT[:], varT[:], 0.0)
                stdT = per.tile([P, DCH, K], F32, tag="stdT")
                nc.scalar.activation(stdT[:], varT[:],
                                     mybir.ActivationFunctionType.Sqrt)

                pnT = per.tile([P, DCH, K], F32, tag="pnT")
                nc.vector.scalar_tensor_tensor(
                    pnT[:], meanT[:], float(NSAMP) / (NSHOT + NSAMP), ss55_sb[:],
                    mybir.AluOpType.mult, mybir.AluOpType.add,
                )
                m3 = per.tile([P, DCH, K], F32, tag="m3")
                nc.vector.tensor_tensor(m3[:], stdT[:], ns55_sb[:],
                                        mybir.AluOpType.mult)
                nc.vector.tensor_tensor(pnT[:], pnT[:], m3[:], mybir.AluOpType.add)

                wN = per.tile([P, DCH, K], F32R, tag="wN")
                nc.vector.tensor_scalar_mul(wN[:], pnT[:], -2.0)
                # ||p_new||^2 row
                nc.vector.tensor_tensor(m2[:], pnT[:], pnT[:], mybir.AluOpType.mult)
                ppn_c = per.tile([P, K], F32, tag="ppn_c")
                nc.vector.tensor_reduce(
                    ppn_c[:],
                    m2[:].rearrange("p c k -> p k c"),
                    mybir.AxisListType.X,
                    mybir.AluOpType.add,
                )
                ppn_ps = p2a.tile([1, K], F32, tag="ppn_ps")
                nc.tensor.matmul(ppn_ps[:], ones_col[:], ppn_c[:],
                                 start=True, stop=True)
                ppn_row = per.tile([1, K], F32R, tag="ppn_row")
                nc.scalar.copy(ppn_row[:], ppn_ps[:])

                # ---------------- phase 2: dist_new + softmax ----------------
                wnr, qTr = wN[:], qT[:]
                oner, ppnr = ones_row[:], ppn_row[:]
                for q0, glen, subs in groups():
                    nsub = len(subs)
                    psD = p2d.tile([K, GQ], F32, tag="psD2")
                    d2 = p2d.tile([P, nsub, K], F32, tag="d2n")
                    for c in range(DCH):
                        nc.tensor.matmul(
                            psD[:, :glen],
                            wnr[:, c, :],
                            qTr[:, c, q0 : q0 + glen],
                            start=(c == 0),
                            stop=False,
                        )
                    nc.tensor.matmul(psD[:, :glen], ppnr, oner[:, :glen],
                                     start=False, stop=True)
                    dT_sb = small.tile([K, GQ], F32, tag="dT2")
                    nc.scalar.copy(dT_sb[:, :glen], psD[:, :glen])
                    for qs, j, tp in subs:
                        nc.tensor.transpose(
                            d2[:tp, j, :],
                            dT_sb[:, j * P : j * P + tp],
                            ident[:K, :K],
                        )

                    dist = small.tile([P, nsub, K], F32, tag="dist2")
                    e_sb = small.tile([P, nsub, K], F32, tag="e2_sb")
                    s2 = small.tile([P, nsub], F32, tag="s2")
                    rs2 = small.tile([P, nsub], F32, tag="rs2")

                    def batch2(pz, jz, tspan):
                        nj = jz.stop - jz.start
                        qqb = (
                            qq_all[pz, tspan]
                            .unsqueeze(2)
                            .broadcast_to([pz.stop - pz.start, nj, K])
                        )
                        nc.vector.scalar_tensor_tensor(
                            dist[pz, jz, :], d2[pz, jz, :], 1.0, qqb,
                            mybir.AluOpType.mult, mybir.AluOpType.add,
                        )
                        nc.scalar.activation(
                            dist[pz, jz, :], dist[pz, jz, :],
                            mybir.ActivationFunctionType.Sqrt,
                        )
                        nc.scalar.activation(
                            e_sb[pz, jz, :], dist[pz, jz, :],
                            mybir.ActivationFunctionType.Exp, scale=-1.0,
                        )
                        nc.vector.tensor_reduce(
                            s2[pz, jz], e_sb[pz, jz, :],
                            mybir.AxisListType.X, mybir.AluOpType.add,
                        )
                        nc.vector.reciprocal(rs2[pz, jz], s2[pz, jz])
                        rsb = (
                            rs2[pz, jz]
                            .unsqueeze(2)
                            .broadcast_to([pz.stop - pz.start, nj, K])
                        )
                        nc.vector.tensor_tensor(
                            y_all[pz, tspan, :], e_sb[pz, jz, :], rsb,
                            mybir.AluOpType.mult,
                        )

                    t0 = q0 // P
                    nfull = sum(1 for (_, _, tp) in subs if tp == P)
                    if nfull:
                        batch2(slice(0, P), slice(0, nfull), slice(t0, t0 + nfull))
                    for qs, j, tp in subs:
                        if tp != P:
                            batch2(slice(0, tp), slice(j, j + 1),
                                   slice(qs // P, qs // P + 1))

                # ---------------- loss partial ----------------
                ls40 = per.tile([P, nt], F32, tag="ls40")
                nc.scalar.activation(ls40[:], s_all[:],
                                     mybir.ActivationFunctionType.Ln)
                u40 = per.tile([P, nt], F32, tag="u40")
                nc.vector.tensor_tensor(u40[:], ls40[:], dl_all[:],
                                        mybir.AluOpType.add)
                lrows = per.tile([P, 1], F32, tag="lrows")
                nc.vector.tensor_reduce(lrows[:], u40[:], mybir.AxisListType.X,
                                        mybir.AluOpType.add)
                lps = p2a.tile([1, 1], F32, tag="lps")
                nc.tensor.matmul(lps[:], ones_col[:], lrows[:],
                                 start=True, stop=True)
                lsb = per.tile([1, 1], F32, tag="lsb")
                nc.scalar.copy(lsb[:], lps[:])

            nc.sync.dma_start(y_out[:], y_all[:].rearrange("p t k -> p (t k)"))
            nc.sync.dma_start(loss_out[:], lsb[:])


def _host_prep(feat, label, noise, qc):
    """Host-side glue: shard queries/labels, replicate tiny proto tensors."""
    feat = np.asarray(feat, dtype=np.float32)
    noise = np.asarray(noise, dtype=np.float32)
    label1 = np.asarray(label)[1].astype(np.int64)

    nt = _ceil_div(qc, P)
    support = feat[:S].reshape(K, NSHOT, D)
    ssum = support.sum(1)                       # [K, D]
    ssq = (support * support).sum(1)            # [K, D]
    proto_e = ssum / NSHOT                      # [K, D]
    ppe = (proto_e * proto_e).sum(-1)           # [K]
    nsum = noise.sum(1)                         # [K, D]

    def tposed(a):  # [K, D] -> [P, DCH*K] with column (c, k)
        return (
            np.ascontiguousarray(
                a.reshape(K, DCH, P).transpose(2, 1, 0).reshape(P, DCH * K)
            ).astype(np.float32)
        )

    shared = {
        "wE": tposed(-2.0 * proto_e),
        "ppE": ppe.reshape(1, K).astype(np.float32),
        "ssumT": tposed(ssum),
        "ssqT": tposed(ssq),
        "ss55T": tposed(ssum / (NSHOT + NSAMP)),
        "nsum55T": tposed(nsum / (NSHOT + NSAMP)),
    }

    queries = feat[S:]
    qn = queries.shape[0]
    in_maps = []
    for c in range(NCORES):
        lo = c * qc
        qshard = np.ascontiguousarray(queries[lo : lo + qc])
        lshard = label1[lo : lo + qc]
        lbl_tiled = np.zeros((P, nt, K), dtype=np.float32)
        idx = np.arange(lshard.shape[0])
        lbl_tiled[idx % P, idx // P, lshard] = 1.0
        in_maps.append(
            {
                "q": qshard,
                "lblhot": lbl_tiled.reshape(P, nt * K),
                **shared,
            }
        )
    assert NCORES * qc == qn
    return in_maps


def _unshard(results, qc):
    nt = _ceil_div(qc, P)
    ys = []
    loss_sum = 0.0
    for c in range(NCORES):
        ytile = results[c]["y"].reshape(P, nt, K).transpose(1, 0, 2)  # [t, p, k]
        ys.append(ytile.reshape(nt * P, K)[:qc])
        loss_sum += float(results[c]["losspart"][0, 0])
    y = np.concatenate(ys, axis=0)
    loss = np.float32(loss_sum / (NCORES * qc))
    return y.astype(np.float32), loss


_CACHE = {}


def _get_program(qc):
    if qc not in _CACHE:
        _CACHE[qc] = _build_program(qc)
    return _CACHE[qc]


def kernel(feat, label, noise):
    qc = (np.asarray(feat).shape[0] - S) // NCORES
    nc = _get_program(qc)
    in_maps = _host_prep(feat, label, noise, qc)
    res = run_bass_kernel_spmd(nc, in_maps, core_ids=list(range(NCORES)))
    return _unshard(res.results, qc)
